# revision 35
# baseline (speedup 1.0000x reference)
"""Trainium2 Bass kernel for nn_Autorec_DG_13116830122688 (AutoRec + GraphConv0D).

Math (reference):
    h   = sigmoid(x @ enc_w.T + enc_b)                      [N, 500]
    agg = segment_sum(h[src] * edge_weight, dst, N)
    hm  = conv_w * agg + (1 - conv_w) * h
    p   = clip(hm @ dec_w.T + dec_b, 1, 5)
    p   = where(ft_n0 == 0 rows, fill, p); where(ft_n1 == 0 cols, fill, p)

Strategy (8 NeuronCores, data-parallel over users):
  - Shard users 2500/core (padded to 2560 = 20x128 tiles).
  - x is transposed and cast to fp16 ON HOST and uploaded item-major, packed
    so each [128 items x 512 users] chunk is one contiguous 128KB DMA.  The
    encoder streams these chunks straight into matmuls against SBUF-resident
    enc_w (bias folded in as an always-one input row) -- no on-device
    transposes at all.  Users are processed in 5 groups of 4 tiles; each
    group owns 4 PSUM accumulator banks for the 47-chunk contraction.
  - ACT sigmoid -> h fp16; chunked AllGather of h overlapped with the
    encoder.  The gathered table is split into an "early" tensor (AG chunks
    0-2, done mid-encoder) and a "late" one (last chunk), so phase-3
    indirect gathers prefetch DURING the encoder; only blocks referencing
    the last chunk wait for the final collective.
  - Message passing + decoder run on COMPACTED live rows only (~80%): rows
    with ft_n0==0 never enter the decoder; their output is written from a
    constant fill tile.  Edges (masked-dst dropped, scaled by conv_w) plus
    explicit self-loop edges with weight (1-conv_w) are grouped by (compact
    dst tile, source dep-group) into 128-edge blocks.  For each block,
    gather h[src] via indirect DMA and multiply with a host-built [128 edges
    x 128 dst] sparse weight matrix on the TensorEngine: aggT += G.T @ W
    accumulates in PSUM in hidden-major layout, feeding the decoder with no
    extra transpose.
  - Decoder: p = hmT.T @ dec_w.T with the column mask baked into host-prepped
    weights plus a constant all-ones hidden unit carrying decoder bias /
    column fill.  DVE clips to [1, 5]; rows are written back to their true
    positions with an indirect row scatter as fp16 (cast to f32 on host).
"""

import os
import sys

import numpy as np

for _p in ("/opt/trn_rl_repo",):
    if _p not in sys.path and os.path.isdir(_p):
        sys.path.insert(0, _p)

# ---- problem constants (hardcoded per contest rules) ----
N_USERS = 20000
N_ITEMS = 6000
HIDDEN = 500
M = 8  # cores
UPC = N_USERS // M  # 2500 users per core
UT = 20  # user tiles per core
UPAD = UT * 128  # 2560
KC = 47  # item chunks of 128 (6016 = 47*128 >= 6001 incl. bias row)
IPAD = KC * 128  # 6016
HPAD = 504  # hidden padded: 4 chunks of 126 (500 real + bias unit)
NG = 5  # encoder user groups
GS = 4  # user tiles per group
GW = GS * 128  # 512 users per group
R_MIN, R_MAX = 1.0, 5.0
CC_TILE_BOUNDS = [4, 10, 16, 20]
NCC = len(CC_TILE_BOUNDS)
# gather-dependency groups over AG chunks: chunks 0-2 all complete well
# before phase 3 so they share one gathered-h tensor; the last chunk gets
# its own so only its (few) blocks wait on the final collective.
DEP_OF_CHUNK = [0, 0, 0, 1]
NDEP = 2
PRE_TILES = 4  # tiles whose dep-0 gathers are issued during phase 1
ENC_SPLIT = [6, 6, 6, 6, 6, 6, 6, 5]  # enc weight load pieces (item chunks)

_PROGRAM_CACHE = {}


def _build_program(S, LT, MT, fill):
    """Build the SPMD Bass program.

    S = per-compact-tile tuple of (dep0, dep1) gather-block counts.
    LT/MT = live/masked compact tile counts. fill = fill constant.
    """
    import concourse.bass as bass
    import concourse.bacc as bacc
    import concourse.mybir as mybir
    from concourse.tile import TileContext

    P = 128
    f32 = mybir.dt.float32
    f16 = mybir.dt.float16
    i32 = mybir.dt.int32
    S_tot = [sum(s) for s in S]
    NBLK = sum(S_tot)
    BOFF = [sum(S_tot[:t]) for t in range(LT)]

    nc = bacc.Bacc(
        "TRN2",
        target_bir_lowering=False,
        debug=False,
        num_devices=M,
        num_swdge_queues=4,
    )

    xt_d = nc.declare_dram_parameter("xt", [NG * KC * P, GW], f16, isOutput=False)
    encw_d = nc.declare_dram_parameter("encw", [P, KC * HIDDEN], f16, isOutput=False)
    decw_d = nc.declare_dram_parameter("decw", [P, 4 * N_ITEMS], f16, isOutput=False)
    si_d = nc.declare_dram_parameter("sidx", [P, NBLK], i32, isOutput=False)
    wb_d = nc.declare_dram_parameter("wblk", [NBLK, P, P], f16, isOutput=False)
    oi_d = nc.declare_dram_parameter("oidx", [P, LT + MT], i32, isOutput=False)
    rv_d = nc.declare_dram_parameter("rowone", [1, P], f16, isOutput=False)
    # row UPC is a dump row for compact-tile padding slots; host drops it
    out_d = nc.declare_dram_parameter("out", [UPC + 1, N_ITEMS], f16, isOutput=True)

    h_loc = nc.dram_tensor("h_loc", [UPAD, HPAD], f16)
    # one gathered-h tensor per DEP GROUP so gathers only wait for the AG
    # chunks they actually reference (group 0's chunks finish mid-encoder)
    crows = []
    lo = 0
    for b in CC_TILE_BOUNDS:
        crows.append((b - lo) * P)
        lo = b
    dep_rows = [0] * NDEP
    ag_dst = []  # (dep group, row offset within its tensor) per AG chunk
    for j in range(NCC):
        d = DEP_OF_CHUNK[j]
        ag_dst.append((d, dep_rows[d]))
        dep_rows[d] += M * crows[j]
    h_fulls = [
        nc.dram_tensor(f"h_full{d}", [dep_rows[d], HPAD], f16, addr_space="Shared")
        for d in range(NDEP)
    ]

    with TileContext(nc) as tc:
        with (
            tc.tile_pool(name="const", bufs=1) as cpool,
            tc.tile_pool(name="xin", bufs=12) as xpool,
            tc.tile_pool(name="hsb", bufs=4) as hpool,
            tc.tile_pool(name="gat", bufs=30) as gpool,
            tc.tile_pool(name="wbl", bufs=30) as wpool,
            tc.tile_pool(name="hmt", bufs=7) as mpool,
            tc.tile_pool(name="pout", bufs=2) as opool,
            tc.tile_pool(name="ps", bufs=1, space="PSUM") as pspool,
        ):
            # encoder weights in small pieces so the first matmuls start
            # early: only piece 0 is DMA'd before the x stream begins; the
            # rest (plus the index tables) are issued a few chunks in, and
            # the decoder weights between encoder groups.
            enc_ts = []
            off = 0
            for i, cnt in enumerate(ENC_SPLIT):
                et = cpool.tile([P, cnt * HIDDEN], f16, tag=f"encw{i}", name=f"encw{i}")
                enc_ts.append((off, et, cnt))
                off += cnt
            nc.sync.dma_start(
                out=enc_ts[0][1][:], in_=encw_d[:, : ENC_SPLIT[0] * HIDDEN]
            )
            si_sb = cpool.tile([P, NBLK], i32, tag="sidx")
            oi_sb = cpool.tile([P, LT + MT], i32, tag="oidx")
            fill_sb = cpool.tile([P, N_ITEMS], f16, tag="fill")
            dec_ts = [
                cpool.tile([P, N_ITEMS], f16, tag=f"decw{i}", name=f"decw{i}")
                for i in range(4)
            ]

            def enc_slice(k):
                for off, et, _ in reversed(enc_ts):
                    if k >= off:
                        return et[:, (k - off) * HIDDEN : (k - off + 1) * HIDDEN]
                raise AssertionError

            # ---- phase-3 gather-block issue helpers ----
            pending = {}

            def issue_blocks(t, dep, cnt0, eng=None):
                # weight-block DMAs go on the sync queue in phase 3 (it is
                # idle there) so they never delay the scalar engine's hmT
                # copies; hoisted phase-1 issues use scalar instead since
                # sync is saturated by the x stream.
                eng = eng or nc.sync
                lst = []
                for s in range(cnt0):
                    b = BOFF[t] + (0 if dep == 0 else S[t][0]) + s
                    gt = gpool.tile([P, HPAD], f16, tag="gt", name="gt")
                    nc.gpsimd.indirect_dma_start(
                        out=gt[:],
                        out_offset=None,
                        in_=h_fulls[dep][:],
                        in_offset=bass.IndirectOffsetOnAxis(
                            ap=si_sb[:, b : b + 1], axis=0
                        ),
                    )
                    wb = wpool.tile([P, P], f16, tag="wb", name="wb")
                    eng.dma_start(out=wb[:], in_=wb_d[b])
                    lst.append((gt, wb))
                return lst

            # ---------------- Phase 1: encoder ----------------
            for g in range(NG):
                accs = []
                for t4 in range(GS):
                    a = pspool.tile([P, 512], f32, tag="acc", bufs=6, name=f"acc{g}_{t4}")
                    accs.append(a)
                for k in range(KC):
                    xk = xpool.tile([P, GW], f16, tag="xk")
                    nc.sync.dma_start(
                        out=xk[:],
                        in_=xt_d[(g * KC + k) * P : (g * KC + k + 1) * P, :],
                    )
                    if g == 0 and k == 0:
                        # next weight pieces on the scalar queue so they
                        # neither delay the first x chunks (issued first on
                        # sync) nor arrive after their first use; the later
                        # pieces follow once the x stream is warm.
                        for off, et, cnt in enc_ts[1:4]:
                            nc.scalar.dma_start(
                                out=et[:],
                                in_=encw_d[:, off * HIDDEN : (off + cnt) * HIDDEN],
                            )
                        nc.vector.memset(fill_sb[:], fill)
                    if g == 0 and k == 12:
                        for off, et, cnt in enc_ts[4:]:
                            nc.scalar.dma_start(
                                out=et[:],
                                in_=encw_d[:, off * HIDDEN : (off + cnt) * HIDDEN],
                            )
                        nc.scalar.dma_start(out=si_sb[:], in_=si_d[:])
                        nc.scalar.dma_start(out=oi_sb[:], in_=oi_d[:])
                    rhs = enc_slice(k)
                    for t4 in range(GS):
                        nc.tensor.matmul(
                            out=accs[t4][:, :HIDDEN],
                            lhsT=xk[:, t4 * P : (t4 + 1) * P],
                            rhs=rhs,
                            start=(k == 0),
                            stop=(k == KC - 1),
                        )
                for t4 in range(GS):
                    ut = g * GS + t4
                    hsb = hpool.tile([P, HPAD], f16, tag="hsb")
                    nc.scalar.activation(
                        out=hsb[:, :HIDDEN],
                        in_=accs[t4][:, :HIDDEN],
                        func=mybir.ActivationFunctionType.Sigmoid,
                    )
                    nc.vector.memset(hsb[:, HIDDEN:HPAD], 0.0)
                    nc.sync.dma_start(
                        out=h_loc[ut * P : (ut + 1) * P, :], in_=hsb[:]
                    )
                    # ---- Phase 2 (interleaved): chunked all-gather ----
                    if (ut + 1) in CC_TILE_BOUNDS:
                        j = CC_TILE_BOUNDS.index(ut + 1)
                        lo = 0 if j == 0 else CC_TILE_BOUNDS[j - 1]
                        hi = CC_TILE_BOUNDS[j]
                        d, roff = ag_dst[j]
                        nc.gpsimd.collective_compute(
                            "AllGather",
                            mybir.AluOpType.bypass,
                            replica_groups=[list(range(M))],
                            ins=[h_loc[lo * P : hi * P, :]],
                            outs=[
                                h_fulls[d][roff : roff + M * (hi - lo) * P, :]
                            ],
                        )
                        if hi == 16:
                            # hoist the first tiles' dep-0 gathers so they run
                            # during the encoder: the gpsimd queue is in-order,
                            # so they must be issued BEFORE the last AG (whose
                            # issue blocks on the final h tiles).
                            for pt in range(PRE_TILES):
                                pending[pt] = issue_blocks(
                                    pt, 0, S[pt][0], eng=nc.scalar
                                )
                if g < 4:  # deferred decoder-weight piece
                    nc.sync.dma_start(
                        out=dec_ts[g][:],
                        in_=decw_d[:, g * N_ITEMS : (g + 1) * N_ITEMS],
                    )

            # ---------------- Phase 3: message passing + decoder ----------------
            # out scatters are deferred 2 tiles so their PSB-ready waits never
            # stall the (in-order) gpsimd queue ahead of later gather issues.
            scatter_q = []

            def flush_scatter(limit):
                while len(scatter_q) > limit:
                    col, psb_ = scatter_q.pop(0)
                    nc.gpsimd.indirect_dma_start(
                        out=out_d[:],
                        out_offset=bass.IndirectOffsetOnAxis(
                            ap=oi_sb[:, col : col + 1], axis=0
                        ),
                        in_=psb_[:],
                        in_offset=None,
                    )

            def mp_pass(agg_ps, blocks):
                # keep each PSUM sub-region's accumulation group contiguous:
                # interleaved start=True matmuls in one bank clobber each
                # other's accumulation state.
                n = len(blocks)
                for c in range(4):
                    for s in range(n):
                        nc.tensor.matmul(
                            out=agg_ps[0:126, c * P : (c + 1) * P],
                            lhsT=blocks[s][0][:, c * 126 : (c + 1) * 126],
                            rhs=blocks[s][1][:],
                            start=(s == 0),
                            stop=(s == n - 1),
                        )

            # ---- pass A: dep0-only aggregation for the hoisted tiles runs
            # right after the encoder, WHILE the final collective is still in
            # flight; their dep1 contribution is added in below.
            hmTs = {}
            for t in range(PRE_TILES):
                agg_ps = pspool.tile([P, 512], f32, tag="agg", bufs=2, name=f"aggA{t}")
                mp_pass(agg_ps, pending.pop(t))
                hmT = mpool.tile([P, 512], f16, tag="hmT", name="hmT")
                nc.scalar.activation(
                    out=hmT[0:126, :],
                    in_=agg_ps[0:126, :],
                    func=mybir.ActivationFunctionType.Copy,
                )
                nc.sync.dma_start(
                    out=hmT[122:123, 3 * P : 4 * P], in_=rv_d[0:1, :]
                )
                hmTs[t] = hmT

            for t in range(LT):
                if t in hmTs:
                    hmT = hmTs.pop(t)
                    b1 = issue_blocks(t, 1, S[t][1])
                    flush_scatter(1)
                    if b1:
                        agg2 = pspool.tile(
                            [P, 512], f32, tag="agg", bufs=2, name=f"aggB{t}"
                        )
                        mp_pass(agg2, b1)
                        # late-chunk contribution lands on rows 0:126 only;
                        # the bias row (122, chunk 3) gets +0 since gathered
                        # h pad columns are zero.
                        nc.vector.tensor_add(
                            out=hmT[0:126, :],
                            in0=hmT[0:126, :],
                            in1=agg2[0:126, :],
                        )
                else:
                    agg_ps = pspool.tile(
                        [P, 512], f32, tag="agg", bufs=2, name=f"agg{t}"
                    )
                    if t not in pending:
                        pending[t] = issue_blocks(t, 0, S[t][0])
                    blocks = pending.pop(t) + issue_blocks(t, 1, S[t][1])
                    flush_scatter(1)
                    mp_pass(agg_ps, blocks)
                    hmT = mpool.tile([P, 512], f16, tag="hmT", name="hmT")
                    nc.scalar.activation(
                        out=hmT[0:126, :],
                        in_=agg_ps[0:126, :],
                        func=mybir.ActivationFunctionType.Copy,
                    )
                    # hidden unit 500 (chunk 3, row 122): decoder-bias unit,
                    # on for every live row (padding slots hit the dump row).
                    nc.sync.dma_start(
                        out=hmT[122:123, 3 * P : 4 * P], in_=rv_d[0:1, :]
                    )
                # masked-row fill scatters, spread through phase 3 (pure
                # output writes with no h dependency)
                if t >= 3 and (t - 3) % 3 == 0 and (t - 3) // 3 < MT:
                    m = (t - 3) // 3
                    nc.gpsimd.indirect_dma_start(
                        out=out_d[:],
                        out_offset=bass.IndirectOffsetOnAxis(
                            ap=oi_sb[:, LT + m : LT + m + 1], axis=0
                        ),
                        in_=fill_sb[:],
                        in_offset=None,
                    )
                psb = opool.tile([P, N_ITEMS], f16, tag="psb")
                for half in range(2):
                    pps = []
                    for nn in range(6):
                        p_ps = pspool.tile(
                            [P, 512], f32, tag="acc", bufs=6, name=f"pps{t}_{half}_{nn}"
                        )
                        pps.append(p_ps)
                    # c outer / n inner: reuses the same stationary hmT chunk
                    # for 6 consecutive matmuls.
                    for c in range(4):
                        for nn in range(6):
                            n = half * 6 + nn
                            nc.tensor.matmul(
                                out=pps[nn][:, :HIDDEN],
                                lhsT=hmT[0:126, c * P : (c + 1) * P],
                                rhs=dec_ts[c][
                                    0:126, n * HIDDEN : (n + 1) * HIDDEN
                                ],
                                start=(c == 0),
                                stop=(c == 3),
                            )
                    for nn in range(6):
                        n = half * 6 + nn
                        nc.vector.tensor_scalar(
                            out=psb[:, n * HIDDEN : (n + 1) * HIDDEN],
                            in0=pps[nn][:, :HIDDEN],
                            scalar1=R_MAX,
                            scalar2=R_MIN,
                            op0=mybir.AluOpType.min,
                            op1=mybir.AluOpType.max,
                        )
                scatter_q.append((t, psb))
            for m in range((LT - 3 + 2) // 3, MT):  # any fill scatters left
                nc.gpsimd.indirect_dma_start(
                    out=out_d[:],
                    out_offset=bass.IndirectOffsetOnAxis(
                        ap=oi_sb[:, LT + m : LT + m + 1], axis=0
                    ),
                    in_=fill_sb[:],
                    in_offset=None,
                )
            flush_scatter(0)

    nc.finalize()
    return nc


def _prep_host(x, edge_index, edge_weight, ft_n0, ft_n1, fill_const,
               enc_w, enc_b, dec_w, dec_b, conv_w):
    """All host-side preprocessing: sharding, weight prep, edge packing."""
    x = np.asarray(x, np.float32)
    src = np.asarray(edge_index[0], np.int64)
    dst = np.asarray(edge_index[1], np.int64)
    w = np.asarray(edge_weight, np.float32)
    ft_n0 = np.asarray(ft_n0)
    ft_n1 = np.asarray(ft_n1)
    fill = float(np.asarray(fill_const))
    conv = float(np.asarray(conv_w))
    enc_w = np.asarray(enc_w, np.float32)
    enc_b = np.asarray(enc_b, np.float32)
    dec_w = np.asarray(dec_w, np.float32)
    dec_b = np.asarray(dec_b, np.float32)

    rowmask = ft_n0 == 0  # rows forced to fill
    colmask = ft_n1 == 0  # cols forced to fill

    # ---- x: fp16, padded, transposed item-major, packed per (group, chunk)
    # so each [128 items x 512 users] encoder chunk is contiguous in DRAM ----
    xp = np.zeros((M, UPAD, IPAD), np.float16)
    xp[:, :UPC, :N_ITEMS] = x.reshape(M, UPC, N_ITEMS)
    xp[:, :, N_ITEMS] = 1.0  # encoder-bias input row (item 6000)
    xt_host = np.ascontiguousarray(
        xp.reshape(M, NG, GW, KC, 128).transpose(0, 1, 3, 4, 2)
    ).reshape(M, NG * KC * 128, GW)

    # ---- encoder weights: [6016, 500] -> [128, 47*500] chunk-major ----
    ewp = np.zeros((IPAD, HIDDEN), np.float32)
    ewp[:N_ITEMS] = enc_w.T
    ewp[N_ITEMS] = enc_b
    enc_host = np.ascontiguousarray(
        ewp.reshape(KC, 128, HIDDEN).transpose(1, 0, 2).reshape(128, KC * HIDDEN)
    ).astype(np.float16)

    # ---- decoder weights with baked column mask / bias unit ----
    dw = dec_w.T.copy()  # [500, 6000]
    dw[:, colmask] = 0.0
    hp = np.zeros((HPAD, N_ITEMS), np.float32)
    hp[:HIDDEN] = dw
    hp[HIDDEN] = np.where(colmask, fill, dec_b)  # bias unit (always-on)
    dec_host = np.zeros((128, 4, N_ITEMS), np.float32)
    dec_host[:126] = hp.reshape(4, 126, N_ITEMS).transpose(1, 0, 2)
    dec_host = np.ascontiguousarray(dec_host.reshape(128, 4 * N_ITEMS)).astype(
        np.float16
    )

    # ---- live-row compaction ----
    rm2 = rowmask.reshape(M, UPC)
    nl = (~rm2).sum(axis=1)
    nm = rm2.sum(axis=1)
    LT = int(np.ceil(nl.max() / 128))
    MT = int(np.ceil(nm.max() / 128)) if nm.max() > 0 else 0
    slot_of = np.full((M, UPC), -1, np.int64)
    oidx_host = np.full((M, 128, LT + MT), UPC, np.int32)  # default: dump row
    for c in range(M):
        li = np.where(~rm2[c])[0]
        mi = np.where(rm2[c])[0]
        slot_of[c, li] = np.arange(len(li))
        liv = np.full(LT * 128, UPC, np.int32)
        liv[: len(li)] = li
        oidx_host[c, :, :LT] = liv.reshape(LT, 128).T
        if MT:
            mv = np.full(MT * 128, UPC, np.int32)
            mv[: len(mi)] = mi
            oidx_host[c, :, LT:] = mv.reshape(MT, 128).T

    # ---- edges: filter masked dst, fold conv_w, append self-loops ----
    keep = ~rowmask[dst]
    src_k = src[keep]
    dst_k = dst[keep]
    w_k = (w[keep] * conv).astype(np.float32)
    core_k = dst_k // UPC
    slot_k = slot_of[core_k, dst_k - core_k * UPC]
    # self-loop edges: live row r -> its own compact slot, weight (1-conv)
    selfs = []
    for c in range(M):
        li = np.where(~rm2[c])[0]
        selfs.append(
            (
                c * UPC + li,  # global src row
                np.full(len(li), c, np.int64),
                slot_of[c, li],
                np.full(len(li), 1.0 - conv, np.float32),
            )
        )
    src_a = np.concatenate([src_k] + [s[0] for s in selfs])
    core_a = np.concatenate([core_k] + [s[1] for s in selfs])
    slot_a = np.concatenate([slot_k] + [s[2] for s in selfs])
    w_a = np.concatenate([w_k] + [s[3] for s in selfs])

    tile_l = slot_a // 128
    din = (slot_a % 128).astype(np.int64)

    # AG chunk / dep group per edge source, and gather row RELATIVE to the
    # dep-group tensor (chunk-major, core-major inside each chunk)
    src_core = src_a // UPC
    src_loc = src_a % UPC
    bounds_rows = np.array([b * 128 for b in CC_TILE_BOUNDS])
    starts_rows = np.concatenate([[0], bounds_rows[:-1]])
    crows = bounds_rows - starts_rows
    dep_of = np.array(DEP_OF_CHUNK)
    chunk_off = np.zeros(NCC, np.int64)  # row offset of chunk in its dep tensor
    dep_rows = [0] * NDEP
    for j in range(NCC):
        d = int(dep_of[j])
        chunk_off[j] = dep_rows[d]
        dep_rows[d] += M * int(crows[j])
    cjs = np.searchsorted(bounds_rows, src_loc, side="right")
    djs = dep_of[cjs]
    gsrc_e = (
        chunk_off[cjs] + src_core * crows[cjs] + (src_loc - starts_rows[cjs])
    ).astype(np.int64)

    # group edges by (core, tile, dep group); per-(tile, dep) block quota is
    # the max over cores so the SPMD program is identical on every core.
    order = np.lexsort((djs, tile_l, core_a))
    src_a, w_a, core_a, tile_l, din, djs, gsrc_e = (
        a[order] for a in (src_a, w_a, core_a, tile_l, din, djs, gsrc_e)
    )
    counts3 = np.bincount(
        (core_a * LT + tile_l) * NDEP + djs, minlength=M * LT * NDEP
    ).reshape(M, LT, NDEP)
    S_tj = np.ceil(counts3.max(axis=0) / 128).astype(np.int64)  # [LT, NDEP]
    if S_tj.sum() == 0:
        S_tj[0, 0] = 1
    S = tuple(tuple(int(v) for v in row) for row in S_tj)
    boff3 = np.zeros((LT, NDEP), np.int64)  # block offset of (t, j)
    flat = S_tj.reshape(-1)
    boff3.reshape(-1)[1:] = np.cumsum(flat)[:-1]
    NBLK = int(flat.sum())

    si_host = np.zeros((M, 128, NBLK), np.int32)
    wblk_host = np.zeros((M, NBLK, 128, 128), np.float32)
    starts = np.zeros(M * LT * NDEP + 1, np.int64)
    np.cumsum(counts3.reshape(-1), out=starts[1:])
    for gidx in range(M * LT * NDEP):
        c, rem = divmod(gidx, LT * NDEP)
        t, j = divmod(rem, NDEP)
        n = int(counts3[c, t, j])
        if n == 0:
            continue
        sl = slice(starts[gidx], starts[gidx] + n)
        nq = int(S_tj[t, j])
        cap = nq * 128
        gi = np.zeros(cap, np.int64)
        wi = np.zeros(cap, np.float32)
        di = np.zeros(cap, np.int64)
        gi[:n] = gsrc_e[sl]
        wi[:n] = w_a[sl]
        di[:n] = din[sl]
        b0 = int(boff3[t, j])
        for q in range(nq):
            blk = slice(q * 128, (q + 1) * 128)
            si_host[c, :, b0 + q] = gi[blk]
            wblk_host[c, b0 + q][np.arange(128), di[blk]] = wi[blk]
    wblk_host = wblk_host.astype(np.float16)

    in_maps = []
    for c in range(M):
        in_maps.append(
            {
                "xt": xt_host[c],
                "encw": enc_host,
                "decw": dec_host,
                "sidx": si_host[c],
                "wblk": wblk_host[c],
                "oidx": oidx_host[c],
                "rowone": np.ones((1, 128), np.float16),
            }
        )
    return (S, LT, MT, fill), in_maps


def _install_ntff_hook_shim():
    """The agent image's antenv lacks axon_hooks; synthesize it so
    run_bass_kernel_spmd(trace=True) can capture NTFF profiles."""
    import types

    if "antenv.axon_hooks" in sys.modules:
        return
    try:
        from trn_agent_boot.trn_boot import _ntff_profile_via_ctypes
    except ImportError:
        return
    hook = _ntff_profile_via_ctypes("/opt/axon/libaxon_pjrt.so")
    mod = types.ModuleType("antenv.axon_hooks")
    mod._hook = hook
    mod.set_axon_ntff_profile_hook = lambda h: setattr(mod, "_hook", h)
    mod.get_axon_ntff_profile_hook = lambda: mod._hook
    sys.modules["antenv.axon_hooks"] = mod
    try:
        import antenv

        antenv.axon_hooks = mod
    except ImportError:
        pass


LAST_EXEC_NS = None
LAST_RESULTS = None


def kernel(x, edge_index, edge_weight, ft_n0, ft_n1, fill_const,
           enc_w, enc_b, dec_w, dec_b, conv_w):
    global LAST_EXEC_NS, LAST_RESULTS
    from concourse.bass_utils import run_bass_kernel_spmd

    key, in_maps = _prep_host(
        x, edge_index, edge_weight, ft_n0, ft_n1, fill_const,
        enc_w, enc_b, dec_w, dec_b, conv_w,
    )

    if key not in _PROGRAM_CACHE:
        _PROGRAM_CACHE[key] = _build_program(*key)
    nc = _PROGRAM_CACHE[key]

    trace = os.environ.get("KERNEL_TRACE", "0") == "1"
    tmpdir = os.environ.get("KERNEL_TRACE_DIR") or None
    if trace:
        _install_ntff_hook_shim()
    res = run_bass_kernel_spmd(
        nc,
        in_maps,
        core_ids=list(range(M)),
        trace=trace,
        tmpdir=tmpdir,
    )
    LAST_EXEC_NS = res.exec_time_ns
    LAST_RESULTS = res
    out = np.concatenate([res.results[c]["out"][:UPC] for c in range(M)], axis=0)
    return np.ascontiguousarray(out.astype(np.float32))


# revision 36
# speedup vs baseline: 1.0443x; 1.0443x over previous
"""Trainium2 Bass kernel for nn_Autorec_DG_13116830122688 (AutoRec + GraphConv0D).

Math (reference):
    h   = sigmoid(x @ enc_w.T + enc_b)                      [N, 500]
    agg = segment_sum(h[src] * edge_weight, dst, N)
    hm  = conv_w * agg + (1 - conv_w) * h
    p   = clip(hm @ dec_w.T + dec_b, 1, 5)
    p   = where(ft_n0 == 0 rows, fill, p); where(ft_n1 == 0 cols, fill, p)

Strategy (8 NeuronCores, data-parallel over users):
  - Shard users 2500/core (padded to 2560 = 20x128 tiles).
  - x is transposed and cast to fp16 ON HOST and uploaded item-major, packed
    so each [128 items x 512 users] chunk is one contiguous 128KB DMA.  The
    encoder streams these chunks straight into matmuls against SBUF-resident
    enc_w (bias folded in as an always-one input row) -- no on-device
    transposes at all.  Users are processed in 5 groups of 4 tiles; each
    group owns 4 PSUM accumulator banks for the 47-chunk contraction.
  - ACT sigmoid -> h fp16; chunked AllGather of h overlapped with the
    encoder.  The gathered table is split into an "early" tensor (AG chunks
    0-2, done mid-encoder) and a "late" one (last chunk), so phase-3
    indirect gathers prefetch DURING the encoder; only blocks referencing
    the last chunk wait for the final collective.
  - Message passing + decoder run on COMPACTED live rows only (~80%): rows
    with ft_n0==0 never enter the decoder; their output is written from a
    constant fill tile.  Edges (masked-dst dropped, scaled by conv_w) plus
    explicit self-loop edges with weight (1-conv_w) are grouped by (compact
    dst tile, source dep-group) into 128-edge blocks.  For each block,
    gather h[src] via indirect DMA and multiply with a host-built [128 edges
    x 128 dst] sparse weight matrix on the TensorEngine: aggT += G.T @ W
    accumulates in PSUM in hidden-major layout, feeding the decoder with no
    extra transpose.
  - Decoder: p = hmT.T @ dec_w.T with the column mask baked into host-prepped
    weights plus a constant all-ones hidden unit carrying decoder bias /
    column fill.  DVE clips to [1, 5]; rows are written back to their true
    positions with an indirect row scatter as fp16 (cast to f32 on host).
"""

import os
import sys

import numpy as np

for _p in ("/opt/trn_rl_repo",):
    if _p not in sys.path and os.path.isdir(_p):
        sys.path.insert(0, _p)

# ---- problem constants (hardcoded per contest rules) ----
N_USERS = 20000
N_ITEMS = 6000
HIDDEN = 500
M = 8  # cores
UPC = N_USERS // M  # 2500 users per core
UT = 20  # user tiles per core
UPAD = UT * 128  # 2560
KC = 47  # item chunks of 128 (6016 = 47*128 >= 6001 incl. bias row)
IPAD = KC * 128  # 6016
HPAD = 504  # hidden padded: 4 chunks of 126 (500 real + bias unit)
NG = 5  # encoder user groups
GS = 4  # user tiles per group
GW = GS * 128  # 512 users per group
R_MIN, R_MAX = 1.0, 5.0
CC_TILE_BOUNDS = [4, 10, 16, 20]
NCC = len(CC_TILE_BOUNDS)
# gather-dependency groups over AG chunks: chunks 0-2 all complete well
# before phase 3 so they share one gathered-h tensor; the last chunk gets
# its own so only its (few) blocks wait on the final collective.
DEP_OF_CHUNK = [0, 0, 0, 1]
NDEP = 2
PRE_TILES = 2  # tiles whose dep-0 gathers are issued during phase 1
ENC_SPLIT = [6, 6, 6, 6, 6, 6, 6, 5]  # enc weight load pieces (item chunks)

_PROGRAM_CACHE = {}


def _build_program(S, LT, MT, fill):
    """Build the SPMD Bass program.

    S = per-compact-tile tuple of (dep0, dep1) gather-block counts.
    LT/MT = live/masked compact tile counts. fill = fill constant.
    """
    import concourse.bass as bass
    import concourse.bacc as bacc
    import concourse.mybir as mybir
    from concourse.tile import TileContext

    P = 128
    f32 = mybir.dt.float32
    f16 = mybir.dt.float16
    i32 = mybir.dt.int32
    S_tot = [sum(s) for s in S]
    NBLK = sum(S_tot)
    BOFF = [sum(S_tot[:t]) for t in range(LT)]

    nc = bacc.Bacc(
        "TRN2",
        target_bir_lowering=False,
        debug=False,
        num_devices=M,
        num_swdge_queues=4,
    )

    xt_d = nc.declare_dram_parameter("xt", [NG * KC * P, GW], f16, isOutput=False)
    encw_d = nc.declare_dram_parameter("encw", [P, KC * HIDDEN], f16, isOutput=False)
    decw_d = nc.declare_dram_parameter("decw", [P, 4 * N_ITEMS], f16, isOutput=False)
    si_d = nc.declare_dram_parameter("sidx", [P, NBLK], i32, isOutput=False)
    wb_d = nc.declare_dram_parameter("wblk", [NBLK, P, P], f16, isOutput=False)
    oi_d = nc.declare_dram_parameter("oidx", [P, LT + MT], i32, isOutput=False)
    rv_d = nc.declare_dram_parameter("rowone", [1, P], f16, isOutput=False)
    # row UPC is a dump row for compact-tile padding slots; host drops it
    out_d = nc.declare_dram_parameter("out", [UPC + 1, N_ITEMS], f16, isOutput=True)

    h_loc = nc.dram_tensor("h_loc", [UPAD, HPAD], f16)
    # one gathered-h tensor per DEP GROUP so gathers only wait for the AG
    # chunks they actually reference (group 0's chunks finish mid-encoder)
    crows = []
    lo = 0
    for b in CC_TILE_BOUNDS:
        crows.append((b - lo) * P)
        lo = b
    dep_rows = [0] * NDEP
    ag_dst = []  # (dep group, row offset within its tensor) per AG chunk
    for j in range(NCC):
        d = DEP_OF_CHUNK[j]
        ag_dst.append((d, dep_rows[d]))
        dep_rows[d] += M * crows[j]
    h_fulls = [
        nc.dram_tensor(f"h_full{d}", [dep_rows[d], HPAD], f16, addr_space="Shared")
        for d in range(NDEP)
    ]

    with TileContext(nc) as tc:
        with (
            tc.tile_pool(name="const", bufs=1) as cpool,
            tc.tile_pool(name="xin", bufs=12) as xpool,
            tc.tile_pool(name="hsb", bufs=4) as hpool,
            tc.tile_pool(name="gat", bufs=30) as gpool,
            tc.tile_pool(name="wbl", bufs=30) as wpool,
            tc.tile_pool(name="hmt", bufs=3) as mpool,
            tc.tile_pool(name="pout", bufs=2) as opool,
            tc.tile_pool(name="ps", bufs=1, space="PSUM") as pspool,
        ):
            # encoder weights in small pieces so the first matmuls start
            # early: only piece 0 is DMA'd before the x stream begins; the
            # rest (plus the index tables) are issued a few chunks in, and
            # the decoder weights between encoder groups.
            enc_ts = []
            off = 0
            for i, cnt in enumerate(ENC_SPLIT):
                et = cpool.tile([P, cnt * HIDDEN], f16, tag=f"encw{i}", name=f"encw{i}")
                enc_ts.append((off, et, cnt))
                off += cnt
            nc.sync.dma_start(
                out=enc_ts[0][1][:], in_=encw_d[:, : ENC_SPLIT[0] * HIDDEN]
            )
            si_sb = cpool.tile([P, NBLK], i32, tag="sidx")
            oi_sb = cpool.tile([P, LT + MT], i32, tag="oidx")
            fill_sb = cpool.tile([P, N_ITEMS], f16, tag="fill")
            dec_ts = [
                cpool.tile([P, N_ITEMS], f16, tag=f"decw{i}", name=f"decw{i}")
                for i in range(4)
            ]

            def enc_slice(k):
                for off, et, _ in reversed(enc_ts):
                    if k >= off:
                        return et[:, (k - off) * HIDDEN : (k - off + 1) * HIDDEN]
                raise AssertionError

            # ---- phase-3 gather-block issue helpers ----
            pending = {}

            def issue_blocks(t, dep, cnt0, eng=None):
                # weight-block DMAs go on the sync queue in phase 3 (it is
                # idle there) so they never delay the scalar engine's hmT
                # copies; hoisted phase-1 issues use scalar instead since
                # sync is saturated by the x stream.
                eng = eng or nc.sync
                lst = []
                for s in range(cnt0):
                    b = BOFF[t] + (0 if dep == 0 else S[t][0]) + s
                    gt = gpool.tile([P, HPAD], f16, tag="gt", name="gt")
                    nc.gpsimd.indirect_dma_start(
                        out=gt[:],
                        out_offset=None,
                        in_=h_fulls[dep][:],
                        in_offset=bass.IndirectOffsetOnAxis(
                            ap=si_sb[:, b : b + 1], axis=0
                        ),
                    )
                    wb = wpool.tile([P, P], f16, tag="wb", name="wb")
                    eng.dma_start(out=wb[:], in_=wb_d[b])
                    lst.append((gt, wb))
                return lst

            # ---------------- Phase 1: encoder ----------------
            for g in range(NG):
                accs = []
                for t4 in range(GS):
                    a = pspool.tile([P, 512], f32, tag="acc", bufs=6, name=f"acc{g}_{t4}")
                    accs.append(a)
                for k in range(KC):
                    xk = xpool.tile([P, GW], f16, tag="xk")
                    nc.sync.dma_start(
                        out=xk[:],
                        in_=xt_d[(g * KC + k) * P : (g * KC + k + 1) * P, :],
                    )
                    if g == 0 and k == 0:
                        # next weight pieces on the scalar queue so they
                        # neither delay the first x chunks (issued first on
                        # sync) nor arrive after their first use; the later
                        # pieces follow once the x stream is warm.
                        for off, et, cnt in enc_ts[1:4]:
                            nc.scalar.dma_start(
                                out=et[:],
                                in_=encw_d[:, off * HIDDEN : (off + cnt) * HIDDEN],
                            )
                        nc.vector.memset(fill_sb[:], fill)
                    if g == 0 and k == 12:
                        for off, et, cnt in enc_ts[4:]:
                            nc.scalar.dma_start(
                                out=et[:],
                                in_=encw_d[:, off * HIDDEN : (off + cnt) * HIDDEN],
                            )
                        nc.scalar.dma_start(out=si_sb[:], in_=si_d[:])
                        nc.scalar.dma_start(out=oi_sb[:], in_=oi_d[:])
                    rhs = enc_slice(k)
                    for t4 in range(GS):
                        nc.tensor.matmul(
                            out=accs[t4][:, :HIDDEN],
                            lhsT=xk[:, t4 * P : (t4 + 1) * P],
                            rhs=rhs,
                            start=(k == 0),
                            stop=(k == KC - 1),
                        )
                for t4 in range(GS):
                    ut = g * GS + t4
                    hsb = hpool.tile([P, HPAD], f16, tag="hsb")
                    nc.scalar.activation(
                        out=hsb[:, :HIDDEN],
                        in_=accs[t4][:, :HIDDEN],
                        func=mybir.ActivationFunctionType.Sigmoid,
                    )
                    nc.vector.memset(hsb[:, HIDDEN:HPAD], 0.0)
                    nc.sync.dma_start(
                        out=h_loc[ut * P : (ut + 1) * P, :], in_=hsb[:]
                    )
                    # ---- Phase 2 (interleaved): chunked all-gather ----
                    if (ut + 1) in CC_TILE_BOUNDS:
                        j = CC_TILE_BOUNDS.index(ut + 1)
                        lo = 0 if j == 0 else CC_TILE_BOUNDS[j - 1]
                        hi = CC_TILE_BOUNDS[j]
                        d, roff = ag_dst[j]
                        nc.gpsimd.collective_compute(
                            "AllGather",
                            mybir.AluOpType.bypass,
                            replica_groups=[list(range(M))],
                            ins=[h_loc[lo * P : hi * P, :]],
                            outs=[
                                h_fulls[d][roff : roff + M * (hi - lo) * P, :]
                            ],
                        )
                        if hi == 16:
                            # hoist the first tiles' dep-0 gathers so they run
                            # during the encoder: the gpsimd queue is in-order,
                            # so they must be issued BEFORE the last AG (whose
                            # issue blocks on the final h tiles).
                            for pt in range(PRE_TILES):
                                pending[pt] = issue_blocks(
                                    pt, 0, S[pt][0], eng=nc.scalar
                                )
                if g < 4:  # deferred decoder-weight piece
                    nc.sync.dma_start(
                        out=dec_ts[g][:],
                        in_=decw_d[:, g * N_ITEMS : (g + 1) * N_ITEMS],
                    )

            # ---------------- Phase 3: message passing + decoder ----------------
            # out scatters are deferred 2 tiles so their PSB-ready waits never
            # stall the (in-order) gpsimd queue ahead of later gather issues.
            scatter_q = []

            def flush_scatter(limit):
                while len(scatter_q) > limit:
                    col, psb_ = scatter_q.pop(0)
                    nc.gpsimd.indirect_dma_start(
                        out=out_d[:],
                        out_offset=bass.IndirectOffsetOnAxis(
                            ap=oi_sb[:, col : col + 1], axis=0
                        ),
                        in_=psb_[:],
                        in_offset=None,
                    )

            for t in range(LT):
                agg_ps = pspool.tile([P, 512], f32, tag="agg", bufs=2, name=f"agg{t}")
                if t not in pending:
                    pending[t] = issue_blocks(t, 0, S[t][0])
                blocks = pending.pop(t) + issue_blocks(t, 1, S[t][1])
                flush_scatter(1)
                # masked-row fill scatters, spread through phase 3 (pure
                # output writes with no h dependency)
                if t >= 3 and (t - 3) % 3 == 0 and (t - 3) // 3 < MT:
                    m = (t - 3) // 3
                    nc.gpsimd.indirect_dma_start(
                        out=out_d[:],
                        out_offset=bass.IndirectOffsetOnAxis(
                            ap=oi_sb[:, LT + m : LT + m + 1], axis=0
                        ),
                        in_=fill_sb[:],
                        in_offset=None,
                    )
                gts = [gw[0] for gw in blocks]
                wbs = [gw[1] for gw in blocks]
                nblk_t = S_tot[t]
                # keep each PSUM sub-region's accumulation group contiguous:
                # interleaved start=True matmuls in one bank clobber each
                # other's accumulation state.
                for c in range(4):
                    for s in range(nblk_t):
                        nc.tensor.matmul(
                            out=agg_ps[0:126, c * P : (c + 1) * P],
                            lhsT=gts[s][:, c * 126 : (c + 1) * 126],
                            rhs=wbs[s][:],
                            start=(s == 0),
                            stop=(s == nblk_t - 1),
                        )
                hmT = mpool.tile([P, 512], f16, tag="hmT")
                nc.scalar.activation(
                    out=hmT[0:126, :],
                    in_=agg_ps[0:126, :],
                    func=mybir.ActivationFunctionType.Copy,
                )
                # hidden unit 500 (chunk 3, row 122): decoder-bias unit, on
                # for every live row (padding slots land in the dump row).
                nc.sync.dma_start(
                    out=hmT[122:123, 3 * P : 4 * P], in_=rv_d[0:1, :]
                )
                psb = opool.tile([P, N_ITEMS], f16, tag="psb")
                for half in range(2):
                    pps = []
                    for nn in range(6):
                        p_ps = pspool.tile(
                            [P, 512], f32, tag="acc", bufs=6, name=f"pps{t}_{half}_{nn}"
                        )
                        pps.append(p_ps)
                    # c outer / n inner: reuses the same stationary hmT chunk
                    # for 6 consecutive matmuls.
                    for c in range(4):
                        for nn in range(6):
                            n = half * 6 + nn
                            nc.tensor.matmul(
                                out=pps[nn][:, :HIDDEN],
                                lhsT=hmT[0:126, c * P : (c + 1) * P],
                                rhs=dec_ts[c][
                                    0:126, n * HIDDEN : (n + 1) * HIDDEN
                                ],
                                start=(c == 0),
                                stop=(c == 3),
                            )
                    for nn in range(6):
                        n = half * 6 + nn
                        nc.vector.tensor_scalar(
                            out=psb[:, n * HIDDEN : (n + 1) * HIDDEN],
                            in0=pps[nn][:, :HIDDEN],
                            scalar1=R_MAX,
                            scalar2=R_MIN,
                            op0=mybir.AluOpType.min,
                            op1=mybir.AluOpType.max,
                        )
                scatter_q.append((t, psb))
            for m in range((LT - 3 + 2) // 3, MT):  # any fill scatters left
                nc.gpsimd.indirect_dma_start(
                    out=out_d[:],
                    out_offset=bass.IndirectOffsetOnAxis(
                        ap=oi_sb[:, LT + m : LT + m + 1], axis=0
                    ),
                    in_=fill_sb[:],
                    in_offset=None,
                )
            flush_scatter(0)

    nc.finalize()
    return nc


def _prep_host(x, edge_index, edge_weight, ft_n0, ft_n1, fill_const,
               enc_w, enc_b, dec_w, dec_b, conv_w):
    """All host-side preprocessing: sharding, weight prep, edge packing."""
    x = np.asarray(x, np.float32)
    src = np.asarray(edge_index[0], np.int64)
    dst = np.asarray(edge_index[1], np.int64)
    w = np.asarray(edge_weight, np.float32)
    ft_n0 = np.asarray(ft_n0)
    ft_n1 = np.asarray(ft_n1)
    fill = float(np.asarray(fill_const))
    conv = float(np.asarray(conv_w))
    enc_w = np.asarray(enc_w, np.float32)
    enc_b = np.asarray(enc_b, np.float32)
    dec_w = np.asarray(dec_w, np.float32)
    dec_b = np.asarray(dec_b, np.float32)

    rowmask = ft_n0 == 0  # rows forced to fill
    colmask = ft_n1 == 0  # cols forced to fill

    # ---- x: fp16, padded, transposed item-major, packed per (group, chunk)
    # so each [128 items x 512 users] encoder chunk is contiguous in DRAM ----
    xp = np.zeros((M, UPAD, IPAD), np.float16)
    xp[:, :UPC, :N_ITEMS] = x.reshape(M, UPC, N_ITEMS)
    xp[:, :, N_ITEMS] = 1.0  # encoder-bias input row (item 6000)
    xt_host = np.ascontiguousarray(
        xp.reshape(M, NG, GW, KC, 128).transpose(0, 1, 3, 4, 2)
    ).reshape(M, NG * KC * 128, GW)

    # ---- encoder weights: [6016, 500] -> [128, 47*500] chunk-major ----
    ewp = np.zeros((IPAD, HIDDEN), np.float32)
    ewp[:N_ITEMS] = enc_w.T
    ewp[N_ITEMS] = enc_b
    enc_host = np.ascontiguousarray(
        ewp.reshape(KC, 128, HIDDEN).transpose(1, 0, 2).reshape(128, KC * HIDDEN)
    ).astype(np.float16)

    # ---- decoder weights with baked column mask / bias unit ----
    dw = dec_w.T.copy()  # [500, 6000]
    dw[:, colmask] = 0.0
    hp = np.zeros((HPAD, N_ITEMS), np.float32)
    hp[:HIDDEN] = dw
    hp[HIDDEN] = np.where(colmask, fill, dec_b)  # bias unit (always-on)
    dec_host = np.zeros((128, 4, N_ITEMS), np.float32)
    dec_host[:126] = hp.reshape(4, 126, N_ITEMS).transpose(1, 0, 2)
    dec_host = np.ascontiguousarray(dec_host.reshape(128, 4 * N_ITEMS)).astype(
        np.float16
    )

    # ---- live-row compaction ----
    rm2 = rowmask.reshape(M, UPC)
    nl = (~rm2).sum(axis=1)
    nm = rm2.sum(axis=1)
    LT = int(np.ceil(nl.max() / 128))
    MT = int(np.ceil(nm.max() / 128)) if nm.max() > 0 else 0
    slot_of = np.full((M, UPC), -1, np.int64)
    oidx_host = np.full((M, 128, LT + MT), UPC, np.int32)  # default: dump row
    for c in range(M):
        li = np.where(~rm2[c])[0]
        mi = np.where(rm2[c])[0]
        slot_of[c, li] = np.arange(len(li))
        liv = np.full(LT * 128, UPC, np.int32)
        liv[: len(li)] = li
        oidx_host[c, :, :LT] = liv.reshape(LT, 128).T
        if MT:
            mv = np.full(MT * 128, UPC, np.int32)
            mv[: len(mi)] = mi
            oidx_host[c, :, LT:] = mv.reshape(MT, 128).T

    # ---- edges: filter masked dst, fold conv_w, append self-loops ----
    keep = ~rowmask[dst]
    src_k = src[keep]
    dst_k = dst[keep]
    w_k = (w[keep] * conv).astype(np.float32)
    core_k = dst_k // UPC
    slot_k = slot_of[core_k, dst_k - core_k * UPC]
    # self-loop edges: live row r -> its own compact slot, weight (1-conv)
    selfs = []
    for c in range(M):
        li = np.where(~rm2[c])[0]
        selfs.append(
            (
                c * UPC + li,  # global src row
                np.full(len(li), c, np.int64),
                slot_of[c, li],
                np.full(len(li), 1.0 - conv, np.float32),
            )
        )
    src_a = np.concatenate([src_k] + [s[0] for s in selfs])
    core_a = np.concatenate([core_k] + [s[1] for s in selfs])
    slot_a = np.concatenate([slot_k] + [s[2] for s in selfs])
    w_a = np.concatenate([w_k] + [s[3] for s in selfs])

    tile_l = slot_a // 128
    din = (slot_a % 128).astype(np.int64)

    # AG chunk / dep group per edge source, and gather row RELATIVE to the
    # dep-group tensor (chunk-major, core-major inside each chunk)
    src_core = src_a // UPC
    src_loc = src_a % UPC
    bounds_rows = np.array([b * 128 for b in CC_TILE_BOUNDS])
    starts_rows = np.concatenate([[0], bounds_rows[:-1]])
    crows = bounds_rows - starts_rows
    dep_of = np.array(DEP_OF_CHUNK)
    chunk_off = np.zeros(NCC, np.int64)  # row offset of chunk in its dep tensor
    dep_rows = [0] * NDEP
    for j in range(NCC):
        d = int(dep_of[j])
        chunk_off[j] = dep_rows[d]
        dep_rows[d] += M * int(crows[j])
    cjs = np.searchsorted(bounds_rows, src_loc, side="right")
    djs = dep_of[cjs]
    gsrc_e = (
        chunk_off[cjs] + src_core * crows[cjs] + (src_loc - starts_rows[cjs])
    ).astype(np.int64)

    # group edges by (core, tile, dep group); per-(tile, dep) block quota is
    # the max over cores so the SPMD program is identical on every core.
    order = np.lexsort((djs, tile_l, core_a))
    src_a, w_a, core_a, tile_l, din, djs, gsrc_e = (
        a[order] for a in (src_a, w_a, core_a, tile_l, din, djs, gsrc_e)
    )
    counts3 = np.bincount(
        (core_a * LT + tile_l) * NDEP + djs, minlength=M * LT * NDEP
    ).reshape(M, LT, NDEP)
    S_tj = np.ceil(counts3.max(axis=0) / 128).astype(np.int64)  # [LT, NDEP]
    if S_tj.sum() == 0:
        S_tj[0, 0] = 1
    S = tuple(tuple(int(v) for v in row) for row in S_tj)
    boff3 = np.zeros((LT, NDEP), np.int64)  # block offset of (t, j)
    flat = S_tj.reshape(-1)
    boff3.reshape(-1)[1:] = np.cumsum(flat)[:-1]
    NBLK = int(flat.sum())

    si_host = np.zeros((M, 128, NBLK), np.int32)
    wblk_host = np.zeros((M, NBLK, 128, 128), np.float32)
    starts = np.zeros(M * LT * NDEP + 1, np.int64)
    np.cumsum(counts3.reshape(-1), out=starts[1:])
    for gidx in range(M * LT * NDEP):
        c, rem = divmod(gidx, LT * NDEP)
        t, j = divmod(rem, NDEP)
        n = int(counts3[c, t, j])
        if n == 0:
            continue
        sl = slice(starts[gidx], starts[gidx] + n)
        nq = int(S_tj[t, j])
        cap = nq * 128
        gi = np.zeros(cap, np.int64)
        wi = np.zeros(cap, np.float32)
        di = np.zeros(cap, np.int64)
        gi[:n] = gsrc_e[sl]
        wi[:n] = w_a[sl]
        di[:n] = din[sl]
        b0 = int(boff3[t, j])
        for q in range(nq):
            blk = slice(q * 128, (q + 1) * 128)
            si_host[c, :, b0 + q] = gi[blk]
            wblk_host[c, b0 + q][np.arange(128), di[blk]] = wi[blk]
    wblk_host = wblk_host.astype(np.float16)

    in_maps = []
    for c in range(M):
        in_maps.append(
            {
                "xt": xt_host[c],
                "encw": enc_host,
                "decw": dec_host,
                "sidx": si_host[c],
                "wblk": wblk_host[c],
                "oidx": oidx_host[c],
                "rowone": np.ones((1, 128), np.float16),
            }
        )
    return (S, LT, MT, fill), in_maps


def _install_ntff_hook_shim():
    """The agent image's antenv lacks axon_hooks; synthesize it so
    run_bass_kernel_spmd(trace=True) can capture NTFF profiles."""
    import types

    if "antenv.axon_hooks" in sys.modules:
        return
    try:
        from trn_agent_boot.trn_boot import _ntff_profile_via_ctypes
    except ImportError:
        return
    hook = _ntff_profile_via_ctypes("/opt/axon/libaxon_pjrt.so")
    mod = types.ModuleType("antenv.axon_hooks")
    mod._hook = hook
    mod.set_axon_ntff_profile_hook = lambda h: setattr(mod, "_hook", h)
    mod.get_axon_ntff_profile_hook = lambda: mod._hook
    sys.modules["antenv.axon_hooks"] = mod
    try:
        import antenv

        antenv.axon_hooks = mod
    except ImportError:
        pass


LAST_EXEC_NS = None
LAST_RESULTS = None


def kernel(x, edge_index, edge_weight, ft_n0, ft_n1, fill_const,
           enc_w, enc_b, dec_w, dec_b, conv_w):
    global LAST_EXEC_NS, LAST_RESULTS
    from concourse.bass_utils import run_bass_kernel_spmd

    key, in_maps = _prep_host(
        x, edge_index, edge_weight, ft_n0, ft_n1, fill_const,
        enc_w, enc_b, dec_w, dec_b, conv_w,
    )

    if key not in _PROGRAM_CACHE:
        _PROGRAM_CACHE[key] = _build_program(*key)
    nc = _PROGRAM_CACHE[key]

    trace = os.environ.get("KERNEL_TRACE", "0") == "1"
    tmpdir = os.environ.get("KERNEL_TRACE_DIR") or None
    if trace:
        _install_ntff_hook_shim()
    res = run_bass_kernel_spmd(
        nc,
        in_maps,
        core_ids=list(range(M)),
        trace=trace,
        tmpdir=tmpdir,
    )
    LAST_EXEC_NS = res.exec_time_ns
    LAST_RESULTS = res
    out = np.concatenate([res.results[c]["out"][:UPC] for c in range(M)], axis=0)
    return np.ascontiguousarray(out.astype(np.float32))
